# revision 1
# baseline (speedup 1.0000x reference)
"""Trainium2 Bass kernel for nn_CrossFusion — polynomial-softmax rewrite.

k_dim = 1 makes the attention scores rank-1: e[s,t] = exp(q_s*k_t), so
    den(q) = sum_m q^m/m! * S_m,   S_m = sum_t k_t^m
    num(q) = sum_m q^m/m! * T_m,   T_m = sum_t v_t k_t^m
With |q*k| <= ~1.2 a low-degree truncation suffices (the num/den truncation
errors cancel in the softmax ratio; measured error is bf16-floored): the whole
[S1,S2] attention collapses to power sums over t plus a per-s Horner loop.

x2 path: load f32 natural halves -> convert bf16 (DVE/ACT) + square -> 64 PE
transposes build a stacked T-form tile xs2 = [x2^T ; (x2^2)^T] ([128, 4096]:
partitions 0:64 hold x2 features, 64:128 the squares; column i <-> row
t = 32p' + cc).  All projections (k0, k1, v-num0, v-num1, nv2, n2) are then
ONE bf16 PE matmul per 512-column chunk with the CBN affine folded into the
weight columns:
    v2.Wv = x2.(A*Wv) + B.Wv ;  ||v2||^2 = x2^2.A^2 + x2.(2AB) + ||B||^2
Chunks pack 4-per-PSUM-bank at partition bases 0/32/64/96; each bank is
copied out whole and de-transposed back to column form with 8 PE transposes
so the power-sum chain runs as ~25 tiny [128,64] DVE ops.
CBN stats mu/E[x^2] accumulate on PE from the natural-form tiles (ap-1
matmuls), then get duplicated onto both partition halves with identity
matmuls.  rsqrt is exp(-0.5*ln(x)) so Ln/Exp is the only ACT table loaded.
x1 (query side) uses an XBAR DMA-transpose bounce (tiny tensors).
Output rows are s = 256b + 2p + eo (b,eo in {0,1}, p partition).
"""
import numpy as np

S = 4096
D = 64
H = 2
NCORES = 8
SSH = S // NCORES   # 512 query rows per core
M = 4               # Taylor degree
EPS_BN = 1e-5

_CACHE = {}


def _build(split=True):
    import concourse.bass as bass
    import concourse.tile as tile
    import concourse.mybir as mybir
    from concourse.masks import make_identity

    f32 = mybir.dt.float32
    bf16 = mybir.dt.bfloat16
    AF = mybir.ActivationFunctionType
    ALU = mybir.AluOpType
    P = 128
    NI = S // 2       # 2048 i-indices (t = 2i + eo)
    HW = NI // 2      # 1024 free-size of each half tile in bf16 T-form

    nc = bass.Bass("TRN2", target_bir_lowering=False, debug=False)

    x1s = nc.dram_tensor("x1s", [SSH, D], f32, kind="ExternalInput")
    x1f = nc.dram_tensor("x1f", [S, D], f32, kind="ExternalInput")
    x2 = nc.dram_tensor("x2", [S, D], f32, kind="ExternalInput")
    Wq = nc.dram_tensor("Wq", [D, H], f32, kind="ExternalInput")
    Wk = nc.dram_tensor("Wk", [D, H], f32, kind="ExternalInput")
    Wv = nc.dram_tensor("Wv", [D, H], f32, kind="ExternalInput")
    Wo = nc.dram_tensor("Wo", [H, 2], f32, kind="ExternalInput")
    bo = nc.dram_tensor("bo", [1, 2], f32, kind="ExternalInput")
    Wg1 = nc.dram_tensor("Wg1", [D, D], f32, kind="ExternalInput")
    Wg2 = nc.dram_tensor("Wg2", [D, D], f32, kind="ExternalInput")
    Wb1 = nc.dram_tensor("Wb1", [D, D], f32, kind="ExternalInput")
    Wb2 = nc.dram_tensor("Wb2", [D, D], f32, kind="ExternalInput")
    y = nc.dram_tensor("y", [SSH, 2], f32, kind="ExternalOutput")

    x1sbf = nc.dram_tensor("x1sbf", [SSH, D], bf16)

    with tile.TileContext(nc) as tc:
        with tc.tile_pool(name="sb", bufs=1) as sb, \
             tc.tile_pool(name="psum", bufs=1, space="PSUM") as psum:

            # ---------------- PSUM (2KB banks) ------------------------------
            TR1 = psum.tile([P, 8 * P], bf16, name="TR1")
            TR2 = psum.tile([P, 8 * P], bf16, name="TR2")
            PP1 = psum.tile([P, 512], f32, name="PP1")
            PP2 = psum.tile([P, 512], f32, name="PP2")
            TP = psum.tile([P, 8 * P], bf16, name="TPn")
            TQ = psum.tile([P, 12], bf16, name="TQn")
            PQm = psum.tile([4, 2 * (SSH // 2)], f32, name="PQm")
            PQ = PQm[0:4, 0:SSH // 2]
            PQ2 = PQm[0:2, SSH // 2:2 * (SSH // 2)]
            SM = psum.tile([P, 160], f32, name="SM")
            h_ps = SM[0:64, 0:1]
            zg_ps = SM[0:64, 1:2]
            zb_ps = SM[0:64, 2:3]
            dg_ps = SM[:, 3:4]
            db_ps = SM[:, 4:5]
            mu_ps = SM[0:64, 5:6]
            msq_ps = SM[0:64, 6:7]
            mu128_ps = SM[:, 7:8]
            msq128_ps = SM[:, 8:9]
            consts_ps = SM[0:1, 9:12]
            coef_ps = SM[0:1, 12:12 + 4 * (M + 1)]
            cb9_ps = SM[:, 64:73]
            cb52_ps = SM[:, 73:73 + 4 * (M + 1)]

            # ---------------- SBUF ------------------------------------------
            x2nat_a = sb.tile([P, 16 * D], f32)    # x2 cols cc<16 (natural)
            x2nat_b = sb.tile([P, 16 * D], f32)    # cc>=16
            xi_a = sb.tile([P, 32 * D], bf16)   # interleaved [x2|x2^2] 64-col pairs
            xi_b = sb.tile([P, 32 * D], bf16)
            xs2 = sb.tile([P, S], bf16)            # stacked T-form [x2T; x2sqT]
            x1snat = sb.tile([P, SSH // 2], f32)
            x1sbn = sb.tile([P, SSH // 2], bf16)
            x1sT = sb.tile([P, SSH // 2], bf16)
            x1sq = sb.tile([P, SSH // 2], bf16)
            x1fbig = sb.tile([P, (S // P) * D], f32)
            wg1_sb = sb.tile([D, D], f32)
            wg2_sb = sb.tile([D, D], f32)
            wb1_sb = sb.tile([D, D], f32)
            wb2_sb = sb.tile([D, D], f32)
            wv2 = sb.tile([P, H], f32)
            worow = sb.tile([1, 4], f32)
            borow = sb.tile([1, 2], f32)
            lhsT1 = sb.tile([P, 6], bf16)
            lhsTq = sb.tile([P, 6], bf16)
            ident = sb.tile([P, P], bf16)
            identf = sb.tile([D, D], f32)
            ones_col = sb.tile([P, 1], f32)
            ones128 = sb.tile([P, P], f32)
            ones_bf = sb.tile([P, 1], bf16)
            ones_row = sb.tile([1, P], f32)
            junk = sb.tile([1, 1], f32)
            eps_col = sb.tile([P, 1], f32)
            mu_sb = sb.tile([D, 1], f32)
            msq_sb = sb.tile([D, 1], f32)
            musq = sb.tile([P, 1], f32)
            mu128s = sb.tile([P, 1], f32)
            msq128s = sb.tile([P, 1], f32)
            dgs = sb.tile([P, 1], f32)
            dbs = sb.tile([P, 1], f32)
            var128 = sb.tile([P, 1], f32)
            lnv = sb.tile([P, 1], f32)
            rs128 = sb.tile([P, 1], f32)
            A128 = sb.tile([P, 1], f32)
            muA = sb.tile([P, 1], f32)
            B128 = sb.tile([P, 1], f32)
            zg_sb = sb.tile([D, 1], f32)
            zb_sb = sb.tile([D, 1], f32)
            h_col = sb.tile([D, 1], f32)
            crow = sb.tile([1, 9], f32)
            constsb = sb.tile([P, 9], f32)
            PPs = sb.tile([P, 1024], bf16)
            C = sb.tile([P, 1024], bf16)
            PQs = sb.tile([4, SSH // 2], bf16)
            PQs2 = sb.tile([2, SSH // 2], bf16)
            C1 = sb.tile([P, 12], f32)
            rsn2 = sb.tile([P, 32], f32)
            lt2 = sb.tile([P, 32], f32)
            rsnv = sb.tile([P, 32], f32)
            lt3 = sb.tile([P, 32], f32)
            rsq1 = sb.tile([P, 4], f32)
            lt1 = sb.tile([P, 4], f32)
            K_all = sb.tile([P, (M + 1) * 2 * 64], bf16)
            R = sb.tile([P, 4 * (M + 1)], f32)
            coefrow = sb.tile([1, 4 * (M + 1)], f32)
            cbrow = sb.tile([P, 4 * (M + 1)], f32)
            qhat = sb.tile([P, 8], f32)
            acc = sb.tile([P, 16], f32)
            rden = sb.tile([P, 8], f32)
            rr = sb.tile([P, 8], f32)
            zt = sb.tile([P, 8], f32)
            t2 = sb.tile([P, 8], f32)
            ez = sb.tile([P, 8], f32)
            sig = sb.tile([P, 8], f32)

            x2r = x2.rearrange("(p cc) d -> p (cc d)", p=P)
            x1sr = x1s.rearrange("(p cc) d -> p (cc d)", p=P)
            x1sbfr = x1sbf.rearrange("(p cc) d -> p (cc d)", p=P)
            vq = x1sbf.rearrange("(i two) d -> i (two d)", two=2)

            # ============ loads (x2 first: it gates the longest chain) ======
            nc.sync.dma_start(x2nat_a[:], x2r[:, 0:16 * D])
            nc.sync.dma_start(x2nat_b[:], x2r[:, 16 * D:32 * D])
            nc.sync.dma_start(x1snat[:], x1sr[:, :])

            nc.vector.memset(junk[:], 0.0)
            nc.scalar.activation(junk[:], junk[:], AF.Exp)  # preload ln/exp
            nc.scalar.dma_start(wg1_sb[:], Wg1[:, :])
            nc.scalar.dma_start(x1fbig[:], x1f.rearrange("(p c) d -> p (c d)", p=P))
            nc.scalar.dma_start(wg2_sb[:], Wg2[:, :])
            nc.scalar.dma_start(wb1_sb[:], Wb1[:, :])
            nc.scalar.dma_start(wb2_sb[:], Wb2[:, :])
            nc.scalar.dma_start(wv2[0:64, :], Wv[:, :])
            nc.scalar.dma_start(wv2[64:128, :], Wv[:, :])

            # static prep (Pool engine work precedes its SWDGE DMAs: the
            # casts write into lhsT tiles, so the zero-memsets must come first)
            nc.vector.memset(ones_col[:], 1.0)
            nc.vector.memset(ones128[:], 1.0)
            nc.vector.memset(ones_bf[:], 1.0)
            nc.vector.memset(ones_row[:], 1.0)
            nc.vector.memset(eps_col[:], EPS_BN)
            make_identity(nc, ident[:])
            make_identity(nc, identf[:])
            nc.gpsimd.memset(lhsTq[:], 0.0)
            nc.gpsimd.memset(lhsT1[:], 0.0)
            nc.gpsimd.memset(lhsTq[0:64, 4:5], 1.0)
            nc.gpsimd.memset(lhsTq[64:128, 5:6], 1.0)
            nc.gpsimd.memset(lhsT1[64:128, 5:6], 1.0)

            # Pool SWDGE: tiny weights (a casting SWDGE DMA is per-element,
            # so only [64,2]-sized tensors go through here)
            nc.gpsimd.dma_start(lhsT1[0:64, 0:2], Wk[:, :])
            nc.gpsimd.dma_start(worow[:], Wo.rearrange("h j -> (h j)").rearrange("(o f) -> o f", o=1))
            nc.gpsimd.dma_start(borow[:], bo[:, :])
            nc.gpsimd.dma_start(lhsTq[0:64, 0:3:2], Wq[:, :])
            nc.gpsimd.dma_start(lhsTq[64:128, 1:4:2], Wq[:, :])

            tc.cur_priority += 3000
            nc.vector.tensor_copy(x1sbn[:], x1snat[:])
            nc.sync.dma_start(x1sbfr[:, :], x1sbn[:])
            nc.sync.dma_start_transpose(x1sT[:], vq[:, :])
            tc.cur_priority -= 3000

            def kslice(m):
                return K_all[:, (2 * m) * 64:(2 * m + 1) * 64]

            def uslice(m):
                return K_all[:, (2 * m + 1) * 64:(2 * m + 2) * 64]

            nc.vector.memset(kslice(0), 1.0)  # k~_0 = 1

            # ============ converts + store + XBAR (a: DVE/SP, b: ACT) =======
            # quarter q covers natural chunks cc in [8q, 8q+8); transpose of
            # chunk cc lands at TR[0:64 | 64:128, 128j:128(j+1)], j = cc%8;
            # xs2 column i = 1024q + 128j + p' maps to t = 32p' + cc.
            with tc.high_priority():
                xav = xi_a[:].rearrange("p (c two d) -> p c two d", two=2, d=D)
                xbv = xi_b[:].rearrange("p (c two d) -> p c two d", two=2, d=D)
                nc.vector.tensor_copy(xav[:, :, 0, :],
                                      x2nat_a[:].rearrange("p (c d) -> p c d", d=D))
                nc.scalar.copy(xbv[:, :, 0, :],
                               x2nat_b[:].rearrange("p (c d) -> p c d", d=D))
                nc.vector.tensor_tensor(out=xav[:, :, 1, :], in0=xav[:, :, 0, :],
                                        in1=xav[:, :, 0, :], op=ALU.mult)
                nc.vector.tensor_tensor(out=xbv[:, :, 1, :], in0=xbv[:, :, 0, :],
                                        in1=xbv[:, :, 0, :], op=ALU.mult)
                for q in range(4):
                    TR = (TR1, TR2)[q % 2]
                    xi = (xi_a, xi_b)[q // 2]
                    for j in range(8):
                        cl = slice((8 * (q % 2) + j) * 2 * D, (8 * (q % 2) + j + 1) * 2 * D)
                        nc.tensor.transpose(TR[:, 128 * j:128 * (j + 1)], xi[:, cl], ident[:])
                    nc.vector.tensor_copy(xs2[:, 1024 * q:1024 * (q + 1)], TR[:])


            # ============ PE stats (ap-1 matmuls are ~free) ================

            for half, xn in enumerate((x2nat_a, x2nat_b)):
                for cc in range(16):
                    nc.tensor.matmul(mu_ps, xn[:, cc * D:(cc + 1) * D], ones_col[:],
                                     start=(half == 0 and cc == 0), stop=(half == 1 and cc == 15))
            for half, xi in enumerate((xi_a, xi_b)):
                for cc in range(16):
                    nc.tensor.matmul(msq_ps, xi[:, (2 * cc + 1) * D:(2 * cc + 2) * D], ones_bf[:],
                                     start=(half == 0 and cc == 0), stop=(half == 1 and cc == 15))
            TCf = S // P
            for c in range(TCf):
                nc.tensor.matmul(h_ps, x1fbig[:, c * D:(c + 1) * D], ones_col[:],
                                 start=(c == 0), stop=(c == TCf - 1))
            nc.scalar.activation(mu_sb[:], mu_ps, AF.Copy, scale=1.0 / S)
            nc.scalar.activation(msq_sb[:], msq_ps, AF.Copy, scale=1.0 / S)
            nc.scalar.activation(h_col[:], h_ps, AF.Copy, scale=1.0 / S)
            nc.tensor.matmul(mu128_ps[0:64, :], identf[:], mu_sb[:], start=True, stop=True)
            nc.tensor.matmul(mu128_ps[64:128, :], identf[:], mu_sb[:], start=True, stop=True)
            nc.tensor.matmul(msq128_ps[0:64, :], identf[:], msq_sb[:], start=True, stop=True)
            nc.tensor.matmul(msq128_ps[64:128, :], identf[:], msq_sb[:], start=True, stop=True)

            # ============ CBN MLPs =========================================
            nc.tensor.matmul(zg_ps, wg1_sb[:], h_col[:], start=True, stop=True)
            nc.tensor.matmul(zb_ps, wb1_sb[:], h_col[:], start=True, stop=True)
            nc.scalar.activation(zg_sb[:], zg_ps, AF.Relu)
            nc.scalar.activation(zb_sb[:], zb_ps, AF.Relu)
            nc.tensor.matmul(dg_ps[0:64, :], wg2_sb[:], zg_sb[:], start=True, stop=True)
            nc.tensor.matmul(dg_ps[64:128, :], wg2_sb[:], zg_sb[:], start=True, stop=True)
            nc.tensor.matmul(db_ps[0:64, :], wb2_sb[:], zb_sb[:], start=True, stop=True)
            nc.tensor.matmul(db_ps[64:128, :], wb2_sb[:], zb_sb[:], start=True, stop=True)

            # ============ A, B, lhsT columns ===============================
            nc.scalar.copy(mu128s[:], mu128_ps)
            nc.vector.tensor_tensor(out=musq[:], in0=mu128s[:], in1=mu128s[:], op=ALU.mult)
            nc.vector.tensor_tensor(out=var128[:], in0=msq128_ps, in1=musq[:], op=ALU.subtract)
            nc.scalar.activation(lnv[:], var128[:], AF.Ln, bias=eps_col[:])
            nc.scalar.activation(rs128[:], lnv[:], AF.Exp, scale=-0.5)
            nc.vector.scalar_tensor_tensor(out=A128[:], in0=dg_ps, scalar=1.0,
                                           in1=rs128[:], op0=ALU.add, op1=ALU.mult)
            nc.vector.tensor_tensor(out=muA[:], in0=mu128s[:], in1=A128[:], op=ALU.mult)
            nc.vector.tensor_tensor(out=B128[:], in0=db_ps, in1=muA[:], op=ALU.subtract)
            for hh in range(H):
                nc.vector.tensor_tensor(out=lhsT1[0:64, 2 + hh:3 + hh],
                                        in0=A128[0:64, :], in1=wv2[0:64, hh:hh + 1], op=ALU.mult)
            nc.vector.scalar_tensor_tensor(out=lhsT1[0:64, 4:5], in0=A128[0:64, :], scalar=2.0,
                                           in1=B128[0:64, :], op0=ALU.mult, op1=ALU.mult)
            nc.vector.tensor_tensor(out=lhsT1[64:128, 4:5], in0=A128[64:128, :],
                                    in1=A128[64:128, :], op=ALU.mult)

            nc.tensor.matmul(consts_ps[:, 0:2], B128[0:64, :], wv2[0:64, :], start=True, stop=True)
            nc.tensor.matmul(consts_ps[:, 2:3], B128[0:64, :], B128[0:64, :], start=True, stop=True)
            nc.scalar.copy(crow[:, 0:3], consts_ps)
            nc.gpsimd.tensor_copy(crow[:, 3:7], worow[:])
            nc.gpsimd.tensor_copy(crow[:, 7:9], borow[:])
            nc.tensor.matmul(cb9_ps, ones_row[:], crow[:], start=True, stop=True)
            nc.scalar.copy(constsb[:], cb9_ps)

            tc.cur_priority += 3000
            nc.vector.tensor_tensor(out=x1sq[:], in0=x1sT[:], in1=x1sT[:], op=ALU.mult)
            tc.cur_priority -= 3000

            # ============ stacked projections ===============================
            # chunk c = 4T + u -> rows 32u..32u+6 of PP{T+1}; after the 4th
            # chunk each bank is copied out whole (rows 6..31 of each 32-row
            # group are pre-zeroed by the memsets above).
            CW = 512
            nc.vector.memset(PP1[:], 0.0)
            nc.vector.memset(PP2[:], 0.0)
            for c in range(8):
                cs = slice(c * CW, (c + 1) * CW)
                pp = (PP1, PP2)[c // 4]
                u = c % 4
                nc.tensor.matmul(pp[32 * u:32 * u + 6, :], lhsT1[:], xs2[:, cs],
                                 start=True, stop=True, tile_position=(0, 32 * u))
            nc.vector.tensor_copy(PPs[:, 0:512], PP1[:])
            nc.scalar.copy(PPs[:, 512:1024], PP2[:])
            tc.cur_priority += 3000
            nc.tensor.matmul(PQ, lhsTq[:, 0:4], x1sT[:], start=True, stop=True)
            nc.tensor.matmul(PQ2, lhsTq[:, 4:6], x1sq[:], start=True, stop=True)
            tc.cur_priority -= 3000

            # ============ de-transpose =====================================
            for g in range(8):
                nc.tensor.transpose(TP[:, 128 * g:128 * (g + 1)],
                                    PPs[:, 128 * g:128 * (g + 1)], ident[:])
            nc.vector.tensor_copy(C[:], TP[:])
            Cv = C[:].rearrange("p (g u q) -> p g u q", g=8, u=4)
            tc.cur_priority += 3000
            nc.vector.tensor_copy(PQs[:], PQ)
            nc.vector.tensor_copy(PQs2[:], PQ2)
            for b in range(2):
                nc.tensor.transpose(TQ[:, 6 * b:6 * b + 4], PQs[:, 128 * b:128 * (b + 1)],
                                    ident[0:4, 0:4])
                nc.tensor.transpose(TQ[:, 6 * b + 4:6 * b + 6], PQs2[:, 128 * b:128 * (b + 1)],
                                    ident[0:2, 0:2])
            nc.vector.tensor_copy(C1[:], TQ)
            C1v = C1[:].rearrange("p (b q) -> p b q", b=2)
            tc.cur_priority -= 3000

            # ============ rsqrt = exp(-0.5 ln) =============================
            lt2v = lt2[:].rearrange("p (g u o) -> p g u o", g=8, o=1)
            lt3v = lt3[:].rearrange("p (g u o) -> p g u o", g=8, o=1)
            nc.scalar.activation(lt2v, Cv[:, :, :, 5:6], AF.Ln)
            nc.scalar.activation(rsn2[:], lt2[:], AF.Exp, scale=-0.5)
            nc.scalar.activation(lt3v, Cv[:, :, :, 4:5], AF.Ln, bias=constsb[:, 2:3])
            nc.scalar.activation(rsnv[:], lt3[:], AF.Exp, scale=-0.5)
            nc.scalar.activation(lt1[:], C1v[:, :, 4:6], AF.Ln)
            nc.scalar.activation(rsq1[:], lt1[:], AF.Exp, scale=-0.5)

            # ============ k^, v^, q^, power chain ==========================
            khat = kslice(1)
            vhat = uslice(0)
            nc.vector.tensor_tensor(
                out=khat.rearrange("p (h g u) -> p g u h", h=2, g=8),
                in0=Cv[:, :, :, 0:2],
                in1=rsn2[:].rearrange("p (g u o) -> p g u o", g=8, o=1)
                    .to_broadcast((P, 8, 4, 2)),
                op=ALU.mult)
            rsnvv = rsnv[:].rearrange("p (g u o) -> p g u o", g=8, o=1)
            for hh in range(H):
                nc.vector.scalar_tensor_tensor(
                    out=vhat[:, 32 * hh:32 * (hh + 1)].rearrange("p (g u o) -> p g u o", g=8, o=1),
                    in0=Cv[:, :, :, 2 + hh:3 + hh], scalar=constsb[:, hh:hh + 1],
                    in1=rsnvv, op0=ALU.add, op1=ALU.mult)
            rsq1v = rsq1[:].rearrange("p (b e) -> p b e", b=2)
            for hh in range(H):
                nc.vector.tensor_tensor(
                    out=qhat[:, 4 * hh:4 * (hh + 1)].rearrange("p (b e) -> p b e", b=2),
                    in0=C1v[:, :, 2 * hh:2 * hh + 2], in1=rsq1v, op=ALU.mult)
            for m in range(1, M + 1):
                if m >= 2:
                    nc.vector.scalar_tensor_tensor(out=kslice(m), in0=kslice(m - 1),
                                                   scalar=1.0 / m, in1=khat,
                                                   op0=ALU.mult, op1=ALU.mult)
                nc.gpsimd.tensor_tensor(out=uslice(m), in0=kslice(m),
                                        in1=vhat, op=ALU.mult)
            GL = 2 * 3  # m=0..2 slices ready before the last u-products land
            nc.vector.reduce_sum(R[:, 0:2 * GL],
                                 K_all[:, 0:GL * 64].rearrange("p (g x) -> p g x", x=32),
                                 axis=mybir.AxisListType.X)
            nc.vector.reduce_sum(R[:, 2 * GL:4 * (M + 1)],
                                 K_all[:, GL * 64:(M + 1) * 2 * 64].rearrange("p (g x) -> p g x", x=32),
                                 axis=mybir.AxisListType.X)
            # all-ones lhsT: one matmul = column sums replicated on all partitions
            nc.tensor.matmul(cb52_ps, ones128[:], R[:], start=True, stop=True)
            nc.scalar.copy(cbrow[:], cb52_ps)

            # ============ Horner (den on DVE, num on Pool, in parallel) ====
            accD = acc[:, 0:8].rearrange("p (h c) -> p h c", h=2)
            accN = acc[:, 8:16].rearrange("p (h c) -> p h c", h=2)
            qx = qhat[:].rearrange("p (h c) -> p h c", h=2)

            def cbden(m):
                return (cbrow[:, 4 * m:4 * m + 2]
                        .rearrange("p (h o) -> p h o", h=2).to_broadcast((P, 2, 4)))

            def cbnum(m):
                return (cbrow[:, 4 * m + 2:4 * m + 4]
                        .rearrange("p (h o) -> p h o", h=2).to_broadcast((P, 2, 4)))

            nc.vector.tensor_copy(accD, cbden(M))
            nc.gpsimd.tensor_copy(accN, cbnum(M))
            for m in range(M - 1, -1, -1):
                nc.vector.tensor_tensor(out=accD, in0=accD, in1=qx, op=ALU.mult)
                nc.vector.tensor_tensor(out=accD, in0=accD, in1=cbden(m), op=ALU.add)
            for m in range(M - 1, -1, -1):
                nc.gpsimd.tensor_tensor(out=accN, in0=accN, in1=qx, op=ALU.mult)
                nc.gpsimd.tensor_tensor(out=accN, in0=accN, in1=cbnum(m), op=ALU.add)

            # ============ epilogue =========================================
            nc.vector.reciprocal(rden[:], acc[:, 0:8])
            nc.vector.tensor_tensor(out=rr[:], in0=acc[:, 8:16], in1=rden[:], op=ALU.mult)
            r0 = rr[:, 0:4].rearrange("p (b e o) -> p b e o", b=2, o=1)
            r1 = rr[:, 4:8].rearrange("p (b e o) -> p b e o", b=2, o=1)
            Zv = zt[:].rearrange("p (b e j) -> p b e j", b=2, e=2)
            for j in range(2):
                nc.vector.tensor_scalar(out=Zv[:, :, :, j:j + 1],
                                        in0=r0,
                                        scalar1=constsb[:, 3 + j:4 + j],
                                        scalar2=constsb[:, 7 + j:8 + j],
                                        op0=ALU.mult, op1=ALU.add)
            for j in range(2):
                nc.vector.scalar_tensor_tensor(out=Zv[:, :, :, j:j + 1],
                                               in0=r1,
                                               scalar=constsb[:, 5 + j:6 + j],
                                               in1=Zv[:, :, :, j:j + 1],
                                               op0=ALU.mult, op1=ALU.add)
            nc.scalar.activation(ez[:], zt[:], AF.Exp, scale=-1.0)
            nc.vector.tensor_scalar_add(ez[:], ez[:], 1.0)
            nc.vector.reciprocal(sig[:], ez[:])
            nc.sync.dma_start(y.rearrange("(b p e) j -> p b e j", b=2, p=P),
                              sig[:].rearrange("p (b e j) -> p b e j", b=2, e=2))

    if split:
        _split_waits(nc, mybir)
    return nc


def _split_waits(nc, mybir, maxw=1):
    """This container's walrus build rejects instructions carrying more than
    ~2 sync-wait commands. Split excess waits onto zero-register-write nops
    inserted just before the instruction on the same engine (same-engine
    program order preserves the wait-before-execute semantics)."""
    ctr = 0
    for bb in nc.m.functions[0].blocks:
        new = []
        for inst in bb.instructions:
            si = inst.sync_info
            if si is not None and si.on_wait and len(si.on_wait) > maxw:
                waits = list(si.on_wait)
                ename = str(inst.engine).split(".")[-1]
                for w in waits[:-maxw]:
                    ctr += 1
                    new.append(mybir.InstRegisterMove(
                        name=f"WS-{ctr}",
                        ins=[mybir.ImmediateValue(kind="imm_value", dtype=mybir.dt.int32, value=0)],
                        outs=[mybir.RegisterAccess(kind="register_access", regref=f"{ename}_zero", dtype=mybir.dt.int32)],
                        engine=inst.engine,
                        sync_info=mybir.SyncInfo(on_wait=[w], on_update=[]),
                    ))
                si.on_wait = waits[-maxw:]
            new.append(inst)
        bb.instructions = new


def _get_program():
    if "nc" not in _CACHE:
        _CACHE["nc"] = _build()
    return _CACHE["nc"]


def kernel(x1, x2, Wq, Wk, Wv, Wo, bo, Wg1, Wg2, Wb1, Wb2):
    from concourse import bass_utils

    nc = _get_program()
    x1s_full = np.ascontiguousarray(x1[0])  # [4096, 64]
    x2s = np.ascontiguousarray(x2[0])

    in_maps = []
    for i in range(NCORES):
        in_maps.append({
            "x1s": np.ascontiguousarray(x1s_full[i * SSH:(i + 1) * SSH]),
            "x1f": x1s_full,
            "x2": x2s,
            "Wq": Wq, "Wk": Wk, "Wv": Wv, "Wo": Wo,
            "bo": np.ascontiguousarray(bo[None, :]),
            "Wg1": Wg1, "Wg2": Wg2, "Wb1": Wb1, "Wb2": Wb2,
        })

    # First execution of a freshly-compiled NEFF occasionally reports a
    # transient device error through the PJRT proxy; a retry succeeds.
    last_err = None
    for attempt in range(3):
        try:
            res = bass_utils.run_bass_kernel_spmd(nc, in_maps, core_ids=list(range(NCORES)))
            out = np.concatenate([res.results[i]["y"] for i in range(NCORES)], axis=0)
            return out.reshape(1, S, 2)
        except Exception as e:  # noqa: BLE001
            last_err = e
            import time
            time.sleep(5)
    raise last_err

